# revision 39
# baseline (speedup 1.0000x reference)
"""DilateAttention (kernel=9, dilation=3, hd=32) on 8 NeuronCores via Bass/Tile.

Inputs  q,k,v: [4, 512, 1, 4096] f32  (B, d, 1, L); d = 16 heads x 32.
Output        [4, 1, 4096, 512] f32  (heads concatenated per token).

Math per (b, h): token n attends keys at n + 3*m - 12, m in 0..8 (zero-padded
outside [0, L)); softmax over the 9 taps (nn.Unfold zero-pad semantics).

Distribution: 64 (b,h) pairs -> 8 per core, packed host-side as 2 "quads" of
4 heads: kq/vq/qq [2, 128, L] f32 (rows 32h..32h+32 = head h).  Host work is
pure slicing/stacking; all FLOPs happen on-device.

Per-core kernel (same SPMD program on all 8 cores, different data):
  Inputs arrive as casting SWDGE DMAs (gpsimd-initiated DMAs cast f32->bf16
  in flight) straight into halo-padded bf16 tiles kb/qb/vb [128, 4192] --
  no staging tiles, no cast ops, and half the DMA wire time of f32.  k/q
  chunks lead, v chunks interleave one step behind (transposes need them
  later).  Constants ride the otherwise output-only SP queue.

  Work is a software pipeline over 40 units = (quad, group-of-4-tiles,
  subpair-of-2-heads), g-major so both subpairs consume each input chunk as
  it lands.  Stages per unit, with explicit pipeline lags chosen so the
  steady state is paced by the ACT engine (exp, 878 ns/unit) alone:
    mask-mm (lag -1, i.e. hoisted one unit early): A[128, 2x512-bank f32
              PSUM] = -240 outside the dilated band via one fp8e4 DoubleRow
              matmul per head (0.5 cyc/col -- half the bf16 cost); exp then
              yields ~1e-19 off-band, so no vector-engine mask is needed.
              Hoisting means each PE period opens directly with mm1s, so A
              is ready ~174 ns earlier for the exp.
    mm1 bf16 (lag 0): S^T[u,t] accumulates per head on top (start=False).
    exp ACT (lag 0): P = exp(S^T/sqrt(32)) for both heads in one op (a
              3-level AP skips the bank-pad columns), PSUM -> SBUF bf16.
    vT (lag 0): one 64-contraction bf16 PE-transpose per tile moves both
              heads' v window into PSUM (Bt); one DVE copy rearranges to
              SBUF slots with a persistent ones-column (softmax denom).
    PV bf16 (lag 2): C[t, slot*33..+33] = sum_u P^T[u,t] vT[u,.] with P as
              weights; lag 2 gives P a full period to land in SBUF so the
              PV ldweights never stalls the in-order PE stream (this was
              the binding recurrence: exp -> P visible -> PV -> next mm1s).
    recip+normalize+output (lag 3, DVE): emitted FIRST in each iteration's
              DVE stream -- their inputs are 3 periods old, so they never
              head-block the in-order DVE queue; the vT copy (whose Bt
              lands mid-period) goes last.  Normalized bf16 rows collect in
              a per-subpair staging tile, DMA'd in pieces on the SP queue
              (final piece 1 group wide to shorten the tail).
  PSUM: A pool bufs=3 (6 banks); Bt and Cp share one rotating bank via
  bitcast views (2 banks).  The 2 flush-PVs at the end write into a spare
  A tile (one per bank) instead, sidestepping the Cp bank's TT recurrence.
  The host reassembles pieces and upcasts (bf16 output costs ~1e-3 extra
  rel err, well inside 2e-2).

  Cost-model timeline: 55659 ns (baseline) -> 51001 ns: fp8 DoubleRow
  masks -7 us PE busy, lag-2 PV + lag-3 DVE emission removed the ~150
  ns/unit cross-engine recurrence stalls, fewer/reordered input chunks cut
  Pool SWDGE overhead ~10 us and the startup stalls.  Steady state is now
  ACT-bound (40 exps x 878 ns back-to-back on the critical path); further
  gains require reducing exp columns (score layout change) or DVE work
  (783 ns/unit: vT-copy 258 + recip 133 + normalize 392), both of which
  cost more elsewhere in this cost model.
"""

import numpy as np

import concourse.bacc as bacc
import concourse.bass as bass
import concourse.mybir as mybir
from concourse.tile import TileContext

B, D, L = 4, 512, 4096
HD = 32
NHEAD = D // HD          # 16
NCORES = 8
NQUAD = 2                # 4 heads per quad, 2 quads per core
NSUB = 2                 # head-pairs per quad
HALO = 12                # dilation * (kernel-1) // 2
TSTEP = 104              # queries per tile = 128 - 2*HALO
SLAB = 128               # keys per tile
NT = (L + TSTEP - 1) // TSTEP         # 40 tiles per (b,h)
G = 4                    # tiles per PSUM group
NG = NT // G             # 10 groups
W = 4192                 # padded SBUF width (12 + 4096 + 12, rounded up)
SCALE = float(HD) ** -0.5
MASKNEG = -240.0         # max finite fp8e4; exp(-240*SCALE) ~ 4e-19 ~ 0

F32 = mybir.dt.float32
BF16 = mybir.dt.bfloat16
FP8 = mybir.dt.float8e4


def _maskneg_np():
    # mask[u, t] = 0 iff key (t0-12+u) is a tap of query (t0+t), i.e.
    # u-t in {0, 3, ..., 24}; else MASKNEG (additive, pre-softmax-scale).
    u = np.arange(128)[:, None]
    t = np.arange(TSTEP)[None, :]
    d = u - t
    band = (d >= 0) & (d <= 24) & (d % 3 == 0)
    m = np.where(band, 0.0, MASKNEG).astype(np.float32)
    return np.tile(m, (1, G))


def _build_program():
    import ml_dtypes

    nc = bacc.Bacc(None, target_bir_lowering=False)
    kq = nc.dram_tensor("kq", [NQUAD, 128, L], F32, kind="ExternalInput")
    vq = nc.dram_tensor("vq", [NQUAD, 128, L], F32, kind="ExternalInput")
    qq = nc.dram_tensor("qq", [NQUAD, 128, L], F32, kind="ExternalInput")
    out = nc.dram_tensor(
        "out", [NQUAD, NSUB, TSTEP, NG * 2 * G * HD], BF16, kind="ExternalOutput"
    )

    # fp8 DoubleRow mask operands: lhsT [128, 2, 128] (plane0 = I, plane1 = 0)
    # and rhs [128, 2, 416] (plane0 = mask, plane1 = 0).  DoubleRow halves the
    # per-column PE cost; mask values 0 / -240 are exact in fp8e4.
    mq = _maskneg_np()  # [128, 416], 0 / MASKNEG
    c8 = np.zeros((128, 256 + 2 * TSTEP * G), dtype=np.float32)
    c8[:, 0:128] = np.eye(128, dtype=np.float32)          # id, DoubleRow plane 0
    c8[:, 256 : 256 + TSTEP * G] = mq                     # mask, DoubleRow plane 0
    const8_dram = nc.inline_tensor(
        c8.astype(ml_dtypes.float8_e4m3), name="const8"
    )
    # I64 replicated in each 64-row block: PE needs fmap and weights to
    # start at the same partition index.
    id64_dram = nc.inline_tensor(
        np.tile(np.eye(64, dtype=np.float32), (2, 1)).astype(ml_dtypes.bfloat16),
        name="id64",
    )

    with TileContext(nc) as tc:
        from contextlib import ExitStack

        with ExitStack() as ctx:
            persist = ctx.enter_context(tc.tile_pool(name="persist", bufs=1))
            NSET = 2
            kb = [persist.tile([128, W], BF16, name=f"kb{s}", tag=f"kb{s}") for s in range(NSET)]
            qb = [persist.tile([128, W], BF16, name=f"qb{s}", tag=f"qb{s}") for s in range(NSET)]
            vb = [persist.tile([128, W], BF16, name=f"vb{s}", tag=f"vb{s}") for s in range(NSET)]
            vTring = [
                persist.tile([128, 33 * 2 * G], BF16, name=f"vT{j}", tag=f"vT{j}")
                for j in range(8)
            ]
            const8_sb = persist.tile(
                [128, 256 + 2 * TSTEP * G], FP8, name="const8_sb", tag="const8_sb"
            )
            id64_sb = persist.tile([128, 64], BF16, name="id64_sb", tag="id64_sb")

            for j in range(8):
                nc.vector.memset(vTring[j][:, 32 :: 33], 1.0)
            # one-time zero inits for pad columns (on DVE: the Pool queue
            # must start the casting input DMAs immediately)
            for s in range(NSET):
                nc.vector.memset(kb[s][:, 0:HALO], 0.0)
                nc.vector.memset(kb[s][:, HALO + L : W], 0.0)
                nc.vector.memset(vb[s][:, 0:HALO], 0.0)
                nc.vector.memset(vb[s][:, HALO + L : W], 0.0)
                nc.vector.memset(qb[s][:, L:W], 0.0)

            # ---- pools ----
            psA = ctx.enter_context(tc.tile_pool(name="psA", bufs=3, space="PSUM"))
            psBC = ctx.enter_context(tc.tile_pool(name="psBC", bufs=2, space="PSUM"))
            spP = ctx.enter_context(tc.tile_pool(name="spP", bufs=8))
            spR = ctx.enter_context(tc.tile_pool(name="spR", bufs=8))
            spS = ctx.enter_context(tc.tile_pool(name="spS", bufs=4))

            def emit_loads(qd):
                """One casting SWDGE DMA per chunk moves each input straight
                from f32 DRAM into the padded bf16 SBUF tiles (gpsimd-
                initiated DMAs cast in flight; no staging, no cast ops).
                k/q chunks lead (QK front), v chunks interleave one step
                behind (transposes)."""
                s = qd % NSET
                if qd == 0:
                    kq_bounds = [0, 512, 1536, 2560, L]
                    v_bounds = [0, 1024, 2048, L]
                else:
                    kq_bounds = [0, 2048, L]
                    v_bounds = [0, 2048, L]
                nch = len(kq_bounds) - 1
                for ci in range(nch):
                    c0, c1 = kq_bounds[ci], kq_bounds[ci + 1]
                    nc.gpsimd.dma_start(
                        kb[s][:, HALO + c0 : HALO + c1], kq[qd, :, c0:c1]
                    )
                    nc.gpsimd.dma_start(qb[s][:, c0:c1], qq[qd, :, c0:c1])
                    if ci < len(v_bounds) - 1:
                        v0, v1 = v_bounds[ci], v_bounds[ci + 1]
                        nc.gpsimd.dma_start(
                            vb[s][:, HALO + v0 : HALO + v1], vq[qd, :, v0:v1]
                        )

            # constants on the sync queue (otherwise outputs-only): their
            # HWDGE is free at t=0
            nc.sync.dma_start(const8_sb[:, :], const8_dram[:, :])
            nc.sync.dma_start(id64_sb[:, :], id64_dram[:, :])

            # Software pipeline over ALL (quad, subpair, group) units:
            # PV/recip/normalize of unit i-1 are emitted after the PE front
            # (mask/mm1/vmm) of unit i, so the in-order PE stream never
            # stalls waiting on exp; the pipeline is NOT reset at subpair or
            # quad boundaries.
            units = [
                (qd, sp, g)
                for qd in range(NQUAD)
                for g in range(NG)
                for sp in range(NSUB)
            ]
            gidx = 0
            stages = {}
            pipe = []   # units awaiting their PV stage (lag 2)
            pipe2 = []  # units awaiting recip/TT/output (lag 3)
            emit_loads(0)

            def emit_mask(A):
                # band mask (start=True): the out-of-band MASKNEG bias via an
                # fp8 DoubleRow matmul (0.5 cyc/col); mm1s accumulate on top.
                # Hoisted one iteration early so mm1s open each PE period.
                for h in range(2):
                    nc.tensor.matmul(
                        A[:, 512 * h : 512 * h + TSTEP * G],
                        const8_sb[:, 0:256].rearrange("p (two m) -> p two m", two=2),
                        const8_sb[:, 256:].rearrange("p (two c) -> p two c", two=2),
                        start=True,
                        stop=False,
                        perf_mode=mybir.MatmulPerfMode.DoubleRow,
                        tile_position=(0, 0),
                        skip_group_check=True,
                    )

            Aq = [psA.tile([128, 1024], F32, name="A")]
            emit_mask(Aq[0])
            for i in range(len(units) + 2):
                if i < len(units):
                    qd, sp, g = units[i]
                    s = qd % NSET
                    if (sp, g) == (0, 0) and qd + 1 < NQUAD:
                        emit_loads(qd + 1)
                    if g == 0:
                        stages[(qd, sp)] = spS.tile(
                            [128, NG * 2 * G * HD], BF16, name="stage"
                        )
                    stage = stages[(qd, sp)]
                    A = Aq.pop(0)
                    for l in range(G):
                        t0 = (G * g + l) * TSTEP
                        for h in range(2):
                            r0 = 64 * sp + 32 * h
                            nc.tensor.matmul(
                                A[:, 512 * h + TSTEP * l : 512 * h + TSTEP * (l + 1)],
                                kb[s][r0 : r0 + 32, t0 : t0 + SLAB],
                                qb[s][r0 : r0 + 32, t0 : t0 + TSTEP],
                                start=False,
                                stop=True,
                                tile_position=(r0, 0),
                                skip_group_check=True,
                            )
                    # v transpose: one 64-contraction fp32 PE transpose
                    # per tile moves both heads' v at once straight from the
                    # f32 tile (no cast); Bt cols (l, h, d) -> vT slots are
                    # (l, h)-major
                    BC = psBC.tile([128, 512], F32, name="BC")
                    Bt = BC[:, 0:128].bitcast(BF16)
                    for l in range(G):
                        t0 = (G * g + l) * TSTEP
                        r0 = 64 * sp
                        nc.tensor.transpose(
                            Bt[:, 64 * l : 64 * (l + 1)],
                            vb[s][r0 : r0 + 64, t0 : t0 + SLAB],
                            id64_sb[r0 : r0 + 64, :],
                        )
                    P = spP.tile([128, 2 * TSTEP * G], BF16, name="P")
                    nc.scalar.activation(
                        P[:, :].rearrange("p (h c) -> p h c", h=2),
                        A[:, :].rearrange("p (h c) -> p h c", h=2)[:, :, 0 : TSTEP * G],
                        mybir.ActivationFunctionType.Exp,
                        bias=0.0,
                        scale=SCALE,
                    )
                    vT = vTring[gidx % 8]
                    gidx += 1
                    pipe.append((P, vT, qd, sp, g, Bt))
                # recip/TT of unit i-3 FIRST on the DVE queue: all their deps
                # are 3 periods old, so they never head-block, and they free
                # the Cp bank / stage rows early
                npop2 = 1 if i < len(units) else len(pipe2)
                for _ in range(min(npop2, len(pipe2))):
                    Cp0, qd0, sp0, g0 = pipe2.pop(0)
                    stage0 = stages[(qd0, sp0)]
                    r = spR.tile([128, 2 * G], F32, name="r")
                    nc.vector.reciprocal_approx_fast(
                        out=r[0:TSTEP, :], in_=Cp0[0:TSTEP, 32 :: 33]
                    )
                    st_ap = stage0[
                        0:TSTEP, 2 * G * HD * g0 : 2 * G * HD * (g0 + 1)
                    ].rearrange("p (s d) -> p s d", s=2 * G)
                    c_ap = Cp0[0:TSTEP, :].rearrange("p (s d) -> p s d", s=2 * G)[:, :, 0:32]
                    r_b = r[0:TSTEP, :]
                    r_ap = bass.AP(
                        tensor=r_b.tensor,
                        offset=r_b.offset,
                        ap=[r_b.ap[0], [1, 2 * G], [0, 32]],
                    )
                    nc.vector.tensor_tensor(st_ap, c_ap, r_ap, op=mybir.AluOpType.mult)
                    # output DMA in pieces per subpair (host reassembles);
                    # the final piece is small so the tail DMA is short
                    CW = 2 * G * HD
                    if g0 == 3:
                        nc.sync.dma_start(
                            out[qd0, sp0, :, 0 : 4 * CW], stage0[0:TSTEP, 0 : 4 * CW]
                        )
                    elif g0 == 6:
                        nc.sync.dma_start(
                            out[qd0, sp0, :, 4 * CW : 7 * CW],
                            stage0[0:TSTEP, 4 * CW : 7 * CW],
                        )
                    elif g0 == 8:
                        nc.sync.dma_start(
                            out[qd0, sp0, :, 7 * CW : 9 * CW],
                            stage0[0:TSTEP, 7 * CW : 9 * CW],
                        )
                    elif g0 == NG - 1:
                        nc.sync.dma_start(
                            out[qd0, sp0, :, 9 * CW :], stage0[0:TSTEP, 9 * CW :]
                        )
                if i == len(units):
                    # flush PVs write into a spare A tile (its banks are long
                    # free), sidestepping the BC bank's TT recurrence
                    Aflush = psA.tile([128, 1024], F32, name="A")
                    fluse = 0
                while (len(pipe) > 2) if i < len(units) else pipe:
                    P0, vT0, qd0, sp0, g0, _ = pipe.pop(0)
                    if i >= len(units):
                        # one PV per PSUM bank of the spare tile
                        Cp = Aflush[:, 512 * fluse : 512 * fluse + 33 * 2 * G]
                        fluse += 1
                    else:
                        Cp = BC[:, 128 : 128 + 33 * 2 * G]
                    for h in range(2):
                        for l in range(G):
                            slot = 2 * l + h
                            nc.tensor.matmul(
                                Cp[0:TSTEP, 33 * slot : 33 * (slot + 1)],
                                P0[:, TSTEP * (G * h + l) : TSTEP * (G * h + l + 1)],
                                vT0[:, 33 * slot : 33 * (slot + 1)],
                                start=True,
                                stop=True,
                            )
                    pipe2.append((Cp, qd0, sp0, g0))
                if i < len(units):
                    # TC after recip/TT: Bt lands mid-period, so it would
                    # head-block the in-order DVE queue if emitted first
                    P_, vT_, _qd, _sp, _g, Bt_ = pipe[-1]
                    nc.vector.tensor_copy(
                        vT_[:, :].rearrange("p (s d) -> p s d", s=2 * G)[:, :, 0:32],
                        Bt_[:, :].rearrange("p (s d) -> p s d", s=2 * G),
                    )
                    if i + 1 < len(units):
                        # masks for unit i+3 at the PE tail of iteration i:
                        # mm1s then open each iteration immediately
                        Anew = psA.tile([128, 1024], F32, name="A")
                        emit_mask(Anew)
                        Aq.append(Anew)
    nc.finalize()
    return nc


_CACHE = {}


def _get_program():
    if "nc" not in _CACHE:
        _CACHE["nc"] = _build_program()
    return _CACHE["nc"]


def make_in_maps(q, k, v):
    """Shard + pack FULL inputs into per-core input maps (host-side data
    movement only)."""
    q = np.ascontiguousarray(np.asarray(q), dtype=np.float32)
    k = np.ascontiguousarray(np.asarray(k), dtype=np.float32)
    v = np.ascontiguousarray(np.asarray(v), dtype=np.float32)
    qr = q.reshape(B * NHEAD, HD, L)
    kr = k.reshape(B * NHEAD, HD, L)
    vr = v.reshape(B * NHEAD, HD, L)

    in_maps = []
    for c in range(NCORES):
        base = c * 8
        # quad qd rows 32h..32h+32 = head base + 4*qd + h
        kqa = kr[base : base + 8].reshape(NQUAD, 128, L)
        vqa = vr[base : base + 8].reshape(NQUAD, 128, L)
        qqa = qr[base : base + 8].reshape(NQUAD, 128, L)
        in_maps.append(
            {
                "kq": np.ascontiguousarray(kqa),
                "vq": np.ascontiguousarray(vqa),
                "qq": np.ascontiguousarray(qqa),
            }
        )
    return in_maps


def assemble_output(results):
    """results: list of 8 per-core dicts with 'out'
    [NQUAD, NSUB, TSTEP, NG*2*G*HD] bf16; token (G*g + l)*TSTEP + t of head
    base + 4*qd + 2*sp + h lives at [qd, sp, t, (g, l, h, d)]."""
    full = np.empty((B * NHEAD, L, HD), dtype=np.float32)
    for c in range(NCORES):
        sc = np.asarray(results[c]["out"]).astype(np.float32)
        sc = sc.reshape(NQUAD, NSUB, TSTEP, NG, G, 2, HD)
        # -> [qd, sp, h, g, l, t, d] -> [bh, pos, d]
        sc = sc.transpose(0, 1, 5, 3, 4, 2, 6).reshape(8, NG * G * TSTEP, HD)
        full[c * 8 : (c + 1) * 8] = sc[:, :L, :]
    full = full.reshape(B, NHEAD, L, HD).transpose(0, 2, 1, 3).reshape(B, L, D)
    return full.reshape(B, 1, L, D)


def kernel(q, k, v):
    from concourse.bass_utils import run_bass_kernel_spmd

    in_maps = make_in_maps(q, k, v)
    nc = _get_program()
    res = run_bass_kernel_spmd(nc, in_maps, core_ids=list(range(NCORES)))
    return assemble_output(res.results)


if __name__ == "__main__":
    rng = np.random.default_rng(0)
    q = rng.standard_normal((B, D, 1, L), dtype=np.float32)
    k = rng.standard_normal((B, D, 1, L), dtype=np.float32)
    v = rng.standard_normal((B, D, 1, L), dtype=np.float32)
    o = kernel(q=q, k=k, v=v)
    print("out", o.shape, o.dtype, float(np.abs(o).max()))

